# revision 17
# baseline (speedup 1.0000x reference)
"""Trainium2 Bass kernel for EvaLinearAttention (nn_EvaLinearAttention_40656160424185).

Strategy: data-parallel over batch B=8 across the 8 NeuronCores (one batch
element per core, no collectives).

Per-core math (x: [N, C], N=4097, C=768, H=12, hd=64):
  qkv = x @ qkv_w.T + bias;  rope on q,k (all tokens but CLS)
  kvT_h = sum_n v_h[n]^T k_roped_h[n]            (pass 1, PSUM-accumulated)
  M_h   = kv_h @ proj_w[:, h].T  -> stacked M [C, C]   (tiny mid phase)
  out   = (q_roped / (hd*N)) @ M + proj_b        (pass 2; attn+proj fused)

Implementation: fp8(e4m3) DoubleRow matmuls for the big qkv projection with
host-side hi/lo error compensation (x = xh+xl exact fp8 pair; W = Wh + Wl,
the xl*Wl cross term dropped). x arrives pre-transposed from the host so no
PE transposes are needed anywhere: k/v come out token-major (for the
token-contracted kvT matmuls) while q is computed directly channel-major
(q^T) via W-stationary DoubleRow matmuls; the rope pair-rotation for q^T
(a cross-partition swap) is realized as a second matmul against a
column-pair-swapped copy of Wq. All on-chip intermediates are bf16; kvT,
M and pass-2 run as plain bf16 matmuls. Scales: x*16, W*32 (fp8 range),
folded back via rope tables (1/512) and proj weights; output is written
bf16 scaled by 2^18 (exact power-of-2, undone on host).
"""

import numpy as np
import ml_dtypes

import concourse.bass as bass  # noqa: F401
import concourse.tile as tile
from concourse import bacc, mybir
from concourse.bass_utils import run_bass_kernel_spmd

F32 = mybir.dt.float32
BF16 = mybir.dt.bfloat16
FP8 = mybir.dt.float8e4
DR = mybir.MatmulPerfMode.DoubleRow

NPF8 = ml_dtypes.float8_e4m3
NPBF = np.dtype(ml_dtypes.bfloat16)

B = 8
N = 4097
NPAD = 4224  # 33 * 128
NT = NPAD // 128
C = 768
H = 12
HD = 64
KC = C // 128  # 6 contraction chunks
NG = 3  # 512-col groups over the 1536 k|v output columns
SW = 32.0  # weight fp8 scale
SX = 16.0  # x fp8 scale
SS = SW * SX  # 512; combined scale carried by qkv psums
OS = 2.0 ** 18  # output scale (exact, undone on host)

_CACHE = {}


def _build_nc():
    nc = bacc.Bacc("TRN2", target_bir_lowering=False, debug=False, num_devices=B)

    x8t = nc.dram_tensor("x8t", [128, NT, KC, 2, 128], FP8, kind="ExternalInput")
    # (hi, hi, lo) packed per (group, chunk) so no 0-stride matmul APs needed
    wkv8 = nc.dram_tensor("wkv8", [128, NG, KC, 3, 512], FP8, kind="ExternalInput")
    wq8 = nc.dram_tensor("wq8", [128, KC, C], FP8, kind="ExternalInput")
    qbrope = nc.dram_tensor("qbrope", [NT, 128, C], BF16, kind="ExternalInput")
    kropes = nc.dram_tensor("kropes", [NT, 128, 128], BF16, kind="ExternalInput")
    qropes = nc.dram_tensor("qropes", [NT, 64, 256], BF16, kind="ExternalInput")
    vb512 = nc.dram_tensor("vb512", [1, C], F32, kind="ExternalInput")
    pb18 = nc.dram_tensor("pb18", [1, C], F32, kind="ExternalInput")
    pw_eff = nc.dram_tensor("pw_eff", [128, KC, C], BF16, kind="ExternalInput")
    out = nc.dram_tensor("out", [NPAD, C], BF16, kind="ExternalOutput")

    with tile.TileContext(nc) as tc:
        with (
            tc.tile_pool(name="const", bufs=1) as const_pool,
            tc.tile_pool(name="wpool", bufs=1) as wpool,
            tc.tile_pool(name="qrs", bufs=1) as qrs_pool,
            tc.tile_pool(name="xin", bufs=4) as xin_pool,
            tc.tile_pool(name="tabs", bufs=4) as tab_pool,
            tc.tile_pool(name="work", bufs=3) as work_pool,
            tc.tile_pool(name="outp", bufs=3) as out_pool,
            tc.tile_pool(name="kvps", bufs=1, space="PSUM") as kv_ps_pool,
            tc.tile_pool(name="qqps", bufs=1, space="PSUM") as qq_ps_pool,
            tc.tile_pool(name="kvtps", bufs=1, space="PSUM") as kvt_ps_pool,
        ):
            # ---- constants / weights resident in SBUF ----
            wkv_sb = wpool.tile([128, NG, KC, 3, 512], FP8)
            wq_sb = wpool.tile([128, KC, C], FP8)
            pw_sb = wpool.tile([128, KC, C], BF16)
            m_sb = wpool.tile([128, KC, C], BF16)

            qrs = qrs_pool.tile([128, NT, C], BF16)

            # x tiles for the first few iterations are prefetched ahead of
            # the weight DMAs sharing the sync queue
            prefetched_x = {}

            def load_x(t):
                x_sb = xin_pool.tile([128, KC, 2, 128], FP8, tag="x8t")
                nc.sync.dma_start(x_sb, x8t.ap()[:, t])
                ktab = tab_pool.tile([128, 128], BF16, tag="ktab")
                nc.scalar.dma_start(ktab, kropes.ap()[t])
                qtab = tab_pool.tile([128, 256], BF16, tag="qtab")
                nc.scalar.dma_start(qtab[0:64, :], qropes.ap()[t])
                nc.scalar.dma_start(qtab[64:128, :], qropes.ap()[t])
                qbr = tab_pool.tile([128, C], BF16, tag="qbr")
                nc.scalar.dma_start(qbr, qbrope.ap()[t])
                return (x_sb, ktab, qtab, qbr)

            for _t in range(3):
                prefetched_x[_t] = load_x(_t)

            # per-group weight DMAs so the first matmuls can start early
            for g in range(NG):
                nc.scalar.dma_start(wkv_sb[:, g], wkv8.ap()[:, g])
            for j in range(KC):
                nc.sync.dma_start(wq_sb[:, j], wq8.ap()[:, j])

            vb_full = const_pool.tile([128, C], F32)
            nc.sync.dma_start(vb_full, vb512.ap().broadcast_to([128, C]))
            pb_full = const_pool.tile([128, C], F32)
            nc.sync.dma_start(pb_full, pb18.ap().broadcast_to([128, C]))

            # persistent kvT accumulator: pairs 0-3 in bank 0 (cols 0:512),
            # pairs 4-5 in bank 1 (cols 512:768, rest junk)
            kvt_ps = kvt_ps_pool.tile([128, 1024], F32, tag="kvt", name="kvt")

            state = {}

            def p1_front(t):
                x_sb, ktab, qtab, qbr = prefetched_x.pop(t, None) or load_x(t)

                # ---- k|v: out[tok, col] += sum_c x^T[c,:].T @ Wkv[c, col]
                # DoubleRow slots: (xh_c, xl_c) x (Wh_c, Wh_c)  [exact x]
                # then (xh_c, xh_c+1) x (Wl_c, Wl_c+1)          [W residual]
                kv_ps = kv_ps_pool.tile([128, 1536], F32, tag="kv")
                for g in range(NG):
                    dst = kv_ps[:, g * 512 : (g + 1) * 512]
                    for c in range(KC):
                        nc.tensor.matmul(
                            dst,
                            x_sb[:, c, :, :],
                            wkv_sb[:, g, c, 0:2, :],
                            start=(c == 0),
                            stop=False,
                            perf_mode=DR,
                        )
                    for cp in range(3):
                        c = 2 * cp
                        nc.tensor.matmul(
                            dst,
                            x_sb[:, c : c + 2, 0, :],
                            wkv_sb[:, g, c : c + 2, 2, :],
                            start=False,
                            stop=(cp == 2),
                            perf_mode=DR,
                        )

                # ---- q^T: out[cq, tok] += Wq[c, cq].T @ x^T[c, tok]
                # DoubleRow slots pair adjacent chunks (hi parts only);
                # qrot comes from a partition-swap DMA later, and the q bias
                # enters via the precomputed qbrope table.
                qq_ps = qq_ps_pool.tile([128, 1536], F32, tag="qq")
                for m in range(KC):
                    dst = qq_ps[:, m * 128 : (m + 1) * 128]
                    for j in range(3):
                        nc.tensor.matmul(
                            dst,
                            wq_sb[:, 2 * j : 2 * j + 2, m * 128 : (m + 1) * 128],
                            x_sb[:, 2 * j : 2 * j + 2, 0, :],
                            start=(m in (0, 4) and j == 0),
                            stop=(j == 2),
                            perf_mode=DR,
                        )
                state[t] = (kv_ps, qq_ps, ktab, qtab, qbr)

            def p1_back(t):
                kv_ps, qq_ps, ktab, qtab, qbr = state.pop(t)
                # ACT evicts PSUM -> SBUF bf16 fast (frees banks for the next
                # tile); DVE table-muls then run 2x from all-bf16 SBUF
                k_sb = work_pool.tile([128, C], BF16, tag="ksb")
                nc.scalar.copy(k_sb, kv_ps[:, 0:768])
                v_sb = work_pool.tile([128, C], BF16, tag="v")
                nc.scalar.copy(v_sb, kv_ps[:, 768:1536])
                nc.gpsimd.tensor_add(v_sb, v_sb, vb_full)
                qt_sb = work_pool.tile([128, C], BF16, tag="qt")
                nc.scalar.copy(qt_sb, qq_ps[:, 0:768])
                # qrot^T = partition-pair swap of q^T (engines cannot cross
                # partitions; a strided SBUF->SBUF DMA can)
                qrt_sb = work_pool.tile([128, C], BF16, tag="qrt")
                qtv = qt_sb.rearrange("(i two) f -> i two f", two=2)
                qrv = qrt_sb.rearrange("(i two) f -> i two f", two=2)
                nc.sync.dma_start(qrv[:, 0], qtv[:, 1])
                nc.sync.dma_start(qrv[:, 1], qtv[:, 0])
                # k-side rope (token-major); tables carry 1/SS
                ck = ktab[:, 0:64].unsqueeze(1).broadcast_to([128, H, 64])
                ske = ktab[:, 64:96].unsqueeze(1).broadcast_to([128, H, 32])
                sko = ktab[:, 96:128].unsqueeze(1).broadcast_to([128, H, 32])
                k1 = work_pool.tile([128, C], BF16, tag="k1")
                nc.vector.tensor_mul(
                    k1.rearrange("p (h d) -> p h d", h=H),
                    k_sb.rearrange("p (h d) -> p h d", h=H),
                    ck,
                )
                k2 = work_pool.tile([128, C], BF16, tag="k2")
                k2p = k2.rearrange("p (h i two) -> p h i two", h=H, two=2)
                ksp = k_sb.rearrange("p (h i two) -> p h i two", h=H, two=2)
                nc.vector.tensor_mul(k2p[:, :, :, 0], ksp[:, :, :, 1], ske)
                nc.vector.tensor_mul(k2p[:, :, :, 1], ksp[:, :, :, 0], sko)
                # q^T rope muls (channel-major; same table for all 6 chunks)
                cq = qtab[:, 0:128].unsqueeze(1).broadcast_to([128, KC, 128])
                sq = qtab[:, 128:256].unsqueeze(1).broadcast_to([128, KC, 128])
                q1 = work_pool.tile([128, C], BF16, tag="q1")
                nc.vector.tensor_mul(
                    q1.rearrange("p (j n) -> p j n", j=KC),
                    qt_sb.rearrange("p (j n) -> p j n", j=KC),
                    cq,
                )
                q2 = work_pool.tile([128, C], BF16, tag="q2")
                nc.vector.tensor_mul(
                    q2.rearrange("p (j n) -> p j n", j=KC),
                    qrt_sb.rearrange("p (j n) -> p j n", j=KC),
                    sq,
                )
                nc.gpsimd.tensor_add(qrs[:, t, :], q1, q2)
                nc.vector.tensor_add(qrs[:, t, :], qrs[:, t, :], qbr)
                # kvT accumulation (bf16, contraction over the 128 tokens)
                for p in range(KC):
                    sl = slice(p * 128, (p + 1) * 128)
                    for ki, ksrc_sb in enumerate((k1, k2)):
                        nc.tensor.matmul(
                            kvt_ps[:, sl],
                            v_sb[:, sl],
                            ksrc_sb[:, sl],
                            start=(t == 0 and ki == 0 and p in (0, 4)),
                            stop=(t == NT - 1 and ki == 1 and p in (3, 5)),
                        )

            for t in range(NT + 1):
                if t < NT:
                    p1_front(t)
                if t == 4:
                    for j in range(KC):
                        nc.scalar.dma_start(pw_sb[:, j], pw_eff.ap()[:, j])
                if t >= 1:
                    p1_back(t - 1)

            # ---- mid: M[d, c] = sum_e kv[h, d, e] * pw_eff[(h,e), c] ----
            kvt_sb = wpool.tile([128, C], BF16)
            nc.vector.tensor_copy(kvt_sb, kvt_ps[:, 0:768])
            kvm = kv_ps_pool.tile([128, 1536], F32, tag="kv")
            qqm = qq_ps_pool.tile([128, 1536], F32, tag="qq")
            for p in range(KC):
                slot = (kvm, qqm)[p % 2][:, 0:768]
                for gi in range(3):
                    gs = slice(gi * 256, (gi + 1) * 256)
                    # pending-zero from start=True covers only the matmul's
                    # own partitions, so each head clears its bank itself
                    st = gi % 2 == 0
                    sp = gi % 2 == 1 or gi == 2
                    nc.tensor.matmul(
                        slot[0:64, gs],
                        kvt_sb[0:64, p * 128 : p * 128 + 64],
                        pw_sb[0:64, p, gs],
                        start=st,
                        stop=sp,
                        tile_position=(0, 0),
                    )
                    nc.tensor.matmul(
                        slot[64:128, gs],
                        kvt_sb[64:128, p * 128 + 64 : p * 128 + 128],
                        pw_sb[64:128, p, gs],
                        start=st,
                        stop=sp,
                        tile_position=(64, 64),
                    )
                nc.scalar.copy(m_sb[:, p, 0:512], slot[:, 0:512])
                nc.scalar.copy(m_sb[:, p, 512:768], slot[:, 512:768])

            # ---- pass 2: out[tok, c] = qr^T.T @ M + pb  (bf16) ----
            p2_state = {}

            def p2_front(t):
                slot = (kvm, qqm, kvt_ps)[t % 3][:, 0:768]
                for gofs, glen in ((0, 512), (512, 256)):
                    dst = slot[:, gofs : gofs + glen]
                    for j in range(KC):
                        nc.tensor.matmul(
                            dst,
                            qrs[:, t, j * 128 : (j + 1) * 128],
                            m_sb[:, j, gofs : gofs + glen],
                            start=(j == 0),
                            stop=(j == KC - 1),
                        )
                p2_state[t] = slot

            def p2_back(t):
                slot = p2_state.pop(t)
                o_sb = out_pool.tile([128, C], BF16, tag="osb")
                nc.vector.tensor_add(o_sb, slot, pb_full)
                nc.sync.dma_start(out.ap()[t * 128 : (t + 1) * 128, :], o_sb)

            for t in range(NT + 1):
                if t < NT:
                    p2_front(t)
                if t >= 1:
                    p2_back(t - 1)

    nc.compile()
    return nc


def _prep_inputs(x, rope, qkv_w, q_bias, v_bias, proj_w, proj_b):
    f = np.float32

    sin = rope[:, :HD].astype(f)
    cos = rope[:, HD:].astype(f)
    cfull = np.zeros((NPAD, HD), f)
    cfull[0] = 1.0
    cfull[1:N] = cos
    sfull = np.zeros((NPAD, HD), f)
    sfull[1:N] = sin

    # k tables (token-major): ck | ske | sko, all carrying 1/SS
    kro = np.zeros((NPAD, 128), f)
    kro[:, 0:64] = cfull / SS
    kro[:, 64:96] = -sfull[:, 0::2] / SS
    kro[:, 96:128] = sfull[:, 1::2] / SS
    kropes = np.ascontiguousarray(kro.reshape(NT, 128, 128).astype(NPBF))

    # q tables (channel-major, transposed): cq^T | sq_signed^T
    sgn = np.tile(np.array([-1.0, 1.0], f), HD // 2)
    qro = np.zeros((NT, 64, 256), f)  # cast to bf16 below
    for t in range(NT):
        qro[t, :, 0:128] = cfull[t * 128 : (t + 1) * 128].T / SS
        qro[t, :, 128:256] = (sfull[t * 128 : (t + 1) * 128] * sgn[None, :HD]).T / SS

    wt = np.ascontiguousarray(qkv_w.T.astype(f))  # [C, 3C]
    Wq, Wkv = wt[:, :C], wt[:, C:]
    perm = np.arange(C).reshape(-1, 2)[:, ::-1].reshape(-1)

    kvh = (Wkv * SW).astype(NPF8)
    kvl = ((Wkv * SW) - kvh.astype(f)).astype(NPF8)
    # [128, NG, KC, 3, 512]: (hi, hi, lo)
    wkv8 = np.empty((128, NG, KC, 3, 512), NPF8)
    hi4 = kvh.reshape(KC, 128, NG, 512)
    lo4 = kvl.reshape(KC, 128, NG, 512)
    wkv8[:, :, :, 0, :] = hi4.transpose(1, 2, 0, 3)
    wkv8[:, :, :, 1, :] = hi4.transpose(1, 2, 0, 3)
    wkv8[:, :, :, 2, :] = lo4.transpose(1, 2, 0, 3)

    def wq_pack(W):
        w8 = (W * SW).astype(NPF8)
        return np.ascontiguousarray(w8.reshape(KC, 128, C).transpose(1, 0, 2))

    # rope-transformed q-bias term: rope(q + qb) = rope(q) + qb*c + rot(qb)*s
    qb = q_bias.astype(f)
    ctok = np.tile(cfull, (1, H))          # [NPAD, C]
    stok = np.tile(sfull, (1, H))
    sgn_c = np.tile(np.array([-1.0, 1.0], f), C // 2)
    qb_term = qb[None, :] * ctok + qb[perm][None, :] * (stok * sgn_c[None, :])
    qbrope_t = np.ascontiguousarray(
        qb_term.T.reshape(KC, 128, NT, 128).transpose(2, 1, 0, 3)
        .reshape(NT, 128, C).astype(NPBF)
    )

    pw = proj_w.T.astype(f) * (OS / (HD * N) / SS)
    pw_eff = np.ascontiguousarray(
        pw.reshape(KC, 128, C).transpose(1, 0, 2).astype(NPBF)
    )

    common = dict(
        wkv8=np.ascontiguousarray(wkv8),
        wq8=wq_pack(Wq),
        qbrope=qbrope_t,
        kropes=kropes,
        qropes=np.ascontiguousarray(qro.astype(NPBF)),
        vb512=np.ascontiguousarray(v_bias.astype(f)[None, :] * SS),
        pb18=np.ascontiguousarray(proj_b.astype(f)[None, :] * OS),
        pw_eff=pw_eff,
    )

    in_maps = []
    for b in range(B):
        xs = np.zeros((NPAD, C), f)
        xs[:N] = x[b] * SX
        xh = xs.astype(NPF8)
        xl = (xs - xh.astype(f)).astype(NPF8)
        stacked = np.stack([xh, xl], axis=0).reshape(2, NT, 128, KC, 128)
        m = dict(common)
        m["x8t"] = np.ascontiguousarray(stacked.transpose(4, 1, 3, 0, 2))
        in_maps.append(m)
    return in_maps


def kernel(x, rope, qkv_w, q_bias, v_bias, proj_w, proj_b, _trace=False):
    x = np.asarray(x, dtype=np.float32)
    rope = np.asarray(rope, dtype=np.float32)
    qkv_w = np.asarray(qkv_w, dtype=np.float32)
    q_bias = np.asarray(q_bias, dtype=np.float32)
    v_bias = np.asarray(v_bias, dtype=np.float32)
    proj_w = np.asarray(proj_w, dtype=np.float32)
    proj_b = np.asarray(proj_b, dtype=np.float32)
    if "nc" not in _CACHE:
        _CACHE["nc"] = _build_nc()
    nc = _CACHE["nc"]
    in_maps = _prep_inputs(x, rope, qkv_w, q_bias, v_bias, proj_w, proj_b)
    res = run_bass_kernel_spmd(nc, in_maps, core_ids=list(range(B)), trace=_trace)
    out = np.stack(
        [res.results[b]["out"][:N].astype(np.float32) for b in range(B)], axis=0
    )
    if _trace:
        _CACHE["last_result"] = res
    return out * np.float32(1.0 / OS)
